# revision 16
# baseline (speedup 1.0000x reference)
"""Trainium2 Bass kernel for local (Gaussian-windowed) attention — v3.

Reference computation (per batch b):
    h = target[b]                                # [D]
    p = sigmoid(tanh(h @ Wp + bp) @ Vp + bv) * S # scalar aligned position
    a = h @ Wa + ba                              # [D]
    x[s, d]  = source[b, s, d] * a[d]
    y[s, :]  = softmax(x[s, :])                  # over feature axis
    w[s, :]  = softmax(y[s, :])                  # double softmax
    g[s]     = exp(-2 * ((s - p) / 50)^2)        # Gaussian window
    out[b,d] = sum_s w[s, d] * g[s] * src[b, s, d]

Sharding: data-parallel over batch (4 per core); weights replicated.

Key optimizations (validated numerically on host, total err ~4e-3 vs the
2e-2 gate):
  - W=192 position window around p (truncation 2.4e-4), value path bf16.
  - second-softmax normalizer s2 = sum_d exp(y_d) with y=softmax(x) is
    provably in [513.0, 513.72] -> replaced by the constant 513.05,
    folded into the Gaussian's exp bias. Kills s2/r2/wv entirely; the
    Gaussian ACT writes bf16 directly into the zero-padded PE lhsT.
  - p-chain in bf16 hi/lo (Wp = W_hi + W_lo, h = h_hi + h_lo; keep the
    three O(2^-9) cross terms -> |dp| = 0.006 positions): 12 cheap bf16
    matmuls instead of 8 fp32 LOW_HIGH passes.
  - window start t0 is taken from the hi*hi product alone (|dp| = 3.3
    positions, window has 16 positions of slack) so window DMAs launch
    ~4us earlier; the precise p (for the Gaussian) finishes in parallel.
  - windows split in half across both HWDGE rings (t-major halves are
    contiguous), then cast fp32->bf16 by SWDGE SBUF->SBUF DMAs (GpSimd
    compute CAST measured 3.6us -- avoid).
  - Vp / bv pre-broadcast to 4 rows on the host; bp/ba applied as bf16
    ones-matmuls (both are zeros here); a broadcast via DRAM DRE
    replication.
  - s1 via one wide DVE reduce per batch; 1/s1 folded into exp2's
    per-partition scale (per j-slot).
  - all 4 context vectors accumulate into ONE [4,512] PSUM via the
    zero-padded lhsT columns -> one ScalarE copy + one 8KB output DMA.
"""

from contextlib import ExitStack

import numpy as np

import concourse.bass as bass
import concourse.tile as tile
from concourse import bacc, mybir
from concourse.bass_utils import run_bass_kernel_spmd
from concourse.masks import make_identity

F32 = mybir.dt.float32
BF16 = mybir.dt.bfloat16
I32 = mybir.dt.int32
AF = mybir.ActivationFunctionType
OP = mybir.AluOpType
ET = mybir.EngineType

N_CORES = 8
B, S, D = 32, 4096, 512
BPC = B // N_CORES          # batches per core
KP = D // 128               # contraction chunks of 128 for D=512
WINDOW = 50.0

W = 160                     # positions kept around p (truncation 2.5e-3)
PW = 80                     # partitions; t-major: part p = pos p, p+80
JW = 2                      # half-window slots
S0_MAX = S - W              # 3904
S2C = 513.05                # second-softmax normalizer (in [513.0,513.72])
LNC = float(-np.log(S2C))

PK1_W = D + BPC             # per-k columns: Wp half + tgt half
PTF_W = D + 1               # vp4 | bv
PTB_W = 2 * D               # bp | ba (bf16)


def _emit(ctx: ExitStack, tc: tile.TileContext, outs, ins):
    nc = tc.nc
    (out,) = outs
    (src, pk1h_d, pk1l_d, ptf_d, ptb_d, pack2_d) = ins

    sb = ctx.enter_context(tc.tile_pool(name="sb", bufs=1))
    ps = ctx.enter_context(tc.tile_pool(name="ps", bufs=1, space="PSUM"))
    dram = ctx.enter_context(tc.tile_pool(name="dram", bufs=1, space="DRAM"))

    def const(name, shape, dtype=F32):
        return sb.tile(shape, dtype, tag=name, name=name)

    # ---- constants (GpSimd) -----------------------------------------
    ones = const("ones", [1, D])
    nc.gpsimd.memset(ones[:], 1.0)
    ones_bf = const("ones_bf", [1, 128], BF16)
    nc.gpsimd.memset(ones_bf[:], 1.0)
    ident4 = const("ident4", [4, 4])
    make_identity(nc, ident4[:])
    iota_i = const("iota_i", [PW, JW], I32)
    nc.gpsimd.iota(iota_i[:], pattern=[[PW, JW]], base=0, channel_multiplier=1)
    io50 = const("io50", [PW, JW])
    nc.gpsimd.tensor_copy(io50[:], iota_i[:])
    nc.gpsimd.tensor_scalar_mul(io50[:], io50[:], 1.0 / WINDOW)
    lnc = const("lnc", [PW, 1])
    nc.gpsimd.memset(lnc[:], LNC)
    wvz = []
    for b in range(BPC):
        t = const(f"wvz{b}", [PW, JW, BPC], BF16)
        nc.gpsimd.memset(t[:], 0.0)
        wvz.append(t)

    # ---- weight DMAs: both HWDGE rings, chunked ----------------------
    pk1h = const("pk1h", [128, KP, PK1_W], BF16)
    pk1l = const("pk1l", [128, KP, PK1_W], BF16)
    ptf = const("ptf", [4, PTF_W])
    ptb = const("ptb", [1, PTB_W], BF16)
    pk2 = const("pk2", [128, KP * D], BF16)
    nc.sync.dma_start(ptb[:], ptb_d[:])
    nc.scalar.dma_start(ptf[:], ptf_d[:])
    nc.sync.dma_start(pk1h[:, 0, :], pk1h_d[:, 0, :])
    nc.scalar.dma_start(pk1h[:, 2, :], pk1h_d[:, 2, :])
    nc.sync.dma_start(pk1h[:, 1, :], pk1h_d[:, 1, :])
    nc.scalar.dma_start(pk1h[:, 3, :], pk1h_d[:, 3, :])
    nc.sync.dma_start(pk2[:, 0 : 2 * D], pack2_d[:, 0 : 2 * D])
    nc.scalar.dma_start(pk2[:, 2 * D : 4 * D], pack2_d[:, 2 * D : 4 * D])
    for k in range(KP):
        eng = nc.sync if k % 2 == 0 else nc.scalar
        eng.dma_start(pk1l[:, k, :], pk1l_d[:, k, :])

    vp4 = ptf[:, 0:D]
    bv_ap = ptf[0:1, D : D + 1]
    bp_row = ptb[:, 0:D]
    ba_row = ptb[:, D : 2 * D]

    # ---- p-chain matmuls (bf16 hi/lo) --------------------------------
    # hp1 = bp + h_hi @ W_hi  (drives the speculative t0)
    psum_hp1 = ps.tile([BPC, D], F32, tag="ps_hp1", name="psum_hp1")
    nc.tensor.matmul(psum_hp1[:], lhsT=ones_bf[:1, :BPC], rhs=bp_row,
                     start=True, stop=False)
    for k in range(KP):
        nc.tensor.matmul(psum_hp1[:], lhsT=pk1h[:, k, D : D + BPC],
                         rhs=pk1h[:, k, 0:D], start=False, stop=(k == KP - 1))
    bvh = const("bvh", [1, 1])
    nc.vector.tensor_scalar_mul(bvh[:], bv_ap, 0.5)

    # ---- speculative t0 chain (from hp1 only; |dp| ~ 3.3 pos) --------
    th_s = const("th_s", [BPC, D])
    nc.scalar.activation(th_s[:], psum_hp1[:], AF.Tanh)
    prod_s = const("prod_s", [BPC, D])
    nc.vector.tensor_tensor(prod_s[:], th_s[:], vp4, op=OP.mult)
    scol_s = const("scol_s", [BPC, 1])
    nc.vector.reduce_sum(scol_s[:], prod_s[:], axis=mybir.AxisListType.X)
    psum_srs = ps.tile([1, BPC], F32, tag="ps_srs", name="psum_srs")
    nc.tensor.transpose(psum_srs[:], scol_s[:], ident4[:])
    th2_s = const("th2_s", [1, BPC])
    nc.scalar.activation(th2_s[:], psum_srs[:], AF.Tanh, bias=bvh[:], scale=0.5)
    # s0 = clamp(trunc(2048*th2 + 2048 - 96), 0, 3904); t08 = [s0 | s0+96]
    cf = const("cf", [1, BPC])
    nc.vector.tensor_scalar(cf[:], th2_s[:], float(S) / 2.0,
                            float(S) / 2.0 - W / 2.0, op0=OP.mult, op1=OP.add)
    nc.vector.tensor_scalar(cf[:], cf[:], 0.0, float(S0_MAX),
                            op0=OP.max, op1=OP.min)
    t08i = const("t08i", [1, BPC], I32)
    nc.vector.tensor_copy(t08i[:], cf[:])  # trunc (x >= 0)
    t0f = const("t0f", [1, BPC])
    nc.vector.tensor_copy(t0f[:], t08i[:])

    _, t0v = nc.values_load_multi_w_load_instructions(
        t08i[:1, 0:BPC], engines=(ET.SP, ET.Activation),
        min_val=0, max_val=S0_MAX, skip_runtime_bounds_check=True)

    # ---- window DMAs: per-batch halves on both rings -----------------
    win_f = []
    for b in range(BPC):
        wf = const(f"win_f{b}", [PW, JW, D])
        eng = nc.sync if b % 2 == 0 else nc.scalar
        eng.dma_start(
            wf[:],
            src[b][bass.ds(t0v[b], W), :].rearrange("(t p) d -> p t d", p=PW))
        win_f.append(wf)

    # hp2 = h_hi @ W_lo + h_lo @ W_hi  (correction for the precise p)
    psum_hp2 = ps.tile([BPC, D], F32, tag="ps_hp2", name="psum_hp2")
    for k in range(KP):
        nc.tensor.matmul(psum_hp2[:], lhsT=pk1h[:, k, D : D + BPC],
                         rhs=pk1l[:, k, 0:D], start=(k == 0), stop=False)
    for k in range(KP):
        nc.tensor.matmul(psum_hp2[:], lhsT=pk1l[:, k, D : D + BPC],
                         rhs=pk1h[:, k, 0:D], start=False, stop=(k == KP - 1))
    # a = ba + h_hi @ Wa
    psum_a = ps.tile([BPC, D], F32, tag="ps_a", name="psum_a")
    nc.tensor.matmul(psum_a[:], lhsT=ones_bf[:1, :BPC], rhs=ba_row,
                     start=True, stop=False)
    for k in range(KP):
        nc.tensor.matmul(psum_a[:], lhsT=pk1h[:, k, D : D + BPC],
                         rhs=pk2[:, k * D : (k + 1) * D],
                         start=False, stop=(k == KP - 1))


    # ---- precise p chain (for the Gaussian) --------------------------
    hp1c = const("hp1c", [BPC, D])
    nc.vector.tensor_copy(hp1c[:], psum_hp1[:])
    thsum = const("thsum", [BPC, D])
    nc.vector.tensor_tensor(thsum[:], hp1c[:], psum_hp2[:], op=OP.add)
    th_p = const("th_p", [BPC, D])
    nc.scalar.activation(th_p[:], thsum[:], AF.Tanh)
    prod_p = const("prod_p", [BPC, D])
    nc.vector.tensor_tensor(prod_p[:], th_p[:], vp4, op=OP.mult)
    scol_p = const("scol_p", [BPC, 1])
    nc.vector.reduce_sum(scol_p[:], prod_p[:], axis=mybir.AxisListType.X)
    psum_srp = ps.tile([1, BPC], F32, tag="ps_srp", name="psum_srp")
    nc.tensor.transpose(psum_srp[:], scol_p[:], ident4[:])
    th2_p = const("th2_p", [1, BPC])
    nc.scalar.activation(th2_p[:], psum_srp[:], AF.Tanh, bias=bvh[:], scale=0.5)
    p50 = const("p50", [1, BPC])
    nc.vector.tensor_scalar(p50[:], th2_p[:], float(S) / WINDOW / 2.0,
                            float(S) / WINDOW / 2.0, op0=OP.mult, op1=OP.add)
    q_row = const("q_row", [1, BPC])
    nc.vector.tensor_scalar_mul(q_row[:], t0f[:], 1.0 / WINDOW)
    nc.vector.tensor_tensor(q_row[:], q_row[:], p50[:], op=OP.subtract)
    psum_q = ps.tile([PW, BPC], F32, tag="ps_q", name="psum_q")
    nc.tensor.matmul(psum_q[:], lhsT=ones[:1, :PW], rhs=q_row[:],
                     start=True, stop=True)
    q_bc = const("q_bc", [PW, BPC])
    nc.vector.tensor_copy(q_bc[:], psum_q[:])

    # a row out to DRAM for the replicated broadcast
    a_sb = const("a_sb", [BPC, D], BF16)
    nc.vector.tensor_copy(a_sb[:], psum_a[:])
    abuf = dram.tile([BPC, D], BF16, tag="abuf", name="abuf")
    nc.gpsimd.dma_start(abuf[:], a_sb[:])
    a_bc = const("a_bc", [128, BPC, D], BF16)
    nc.gpsimd.dma_start(
        a_bc[:],
        abuf[:].rearrange("b d -> (b d)")[None, :].to_broadcast((128, BPC * D)))



    # ---- main loop (staggered wavefront over batches) ----------------
    psum_ctx = ps.tile([BPC, D], F32, tag="ps_ctx", name="psum_ctx")
    st = {}

    def stage_a(b):
        # gaussian lhsT column: exp(-2*(io50+q)^2 + ln(1/s2c)) as bf16
        ut = const(f"ut{b}", [PW, JW])
        nc.gpsimd.tensor_scalar_add(ut[:], io50[:], q_bc[:, b : b + 1])
        nc.gpsimd.tensor_tensor(ut[:], ut[:], ut[:], op=OP.mult)
        nc.scalar.activation(wvz[b][:, :, b], ut[:], AF.Exp,
                             scale=-2.0, bias=lnc[:])

    def stage_b(b):
        x = const(f"x{b}", [PW, JW, D], BF16)
        for j in range(JW):
            nc.vector.tensor_tensor(x[:, j, :], win_f[b][:, j, :],
                                    a_bc[0:PW, b, :], op=OP.mult)
        st[b] = (x,)

    def stage_c(b):
        (x,) = st[b]
        e1 = const(f"e1_{b}", [PW, JW, D], BF16)
        nc.scalar.activation(e1[:].rearrange("p t d -> p (t d)"),
                             x[:].rearrange("p t d -> p (t d)"), AF.Exp)
        st[b] = (e1,)

    def stage_d(b):
        (e1,) = st[b]
        s1 = const(f"s1_{b}", [PW, JW])
        nc.vector.reduce_sum(s1[:], e1[:], axis=mybir.AxisListType.X)
        r1 = const(f"r1_{b}", [PW, JW])
        nc.vector.reciprocal(r1[:], s1[:])
        st[b] = (e1, r1)

    def stage_e(b):
        e1, r1 = st[b]
        e2 = const(f"e2_{b}", [PW, JW, D], BF16)
        for j in range(JW):
            nc.scalar.activation(e2[:, j, :], e1[:, j, :], AF.Exp,
                                 scale=r1[:, j : j + 1])
        st[b] = (e2,)

    def stage_f(b):
        (e2,) = st[b]
        t2 = const(f"t2_{b}", [PW, JW, D], BF16)
        nc.vector.tensor_tensor(t2[:], e2[:], win_f[b][:], op=OP.mult)
        for j in range(JW):
            nc.tensor.matmul(psum_ctx[:], lhsT=wvz[b][:, j, :],
                             rhs=t2[:, j, :],
                             start=(b == 0 and j == 0),
                             stop=(b == BPC - 1 and j == JW - 1))

    stages = [stage_a, stage_b, stage_c, stage_d, stage_e, stage_f]
    for wave in range(len(stages) + BPC - 1):
        for b in range(BPC):
            si = wave - b
            if 0 <= si < len(stages):
                stages[si](b)

    out_sb = const("out_sb", [BPC, D])
    nc.scalar.copy(out_sb[:], psum_ctx[:])
    nc.sync.dma_start(out[:], out_sb[:])


def build_nc(sparse: bool = True):
    nc = bacc.Bacc("TRN2", target_bir_lowering=False, debug=False,
                   num_devices=N_CORES)
    src = nc.dram_tensor("source", [BPC, S, D], F32, kind="ExternalInput").ap()
    pk1h = nc.dram_tensor("pack1h", [128, KP, PK1_W], BF16,
                          kind="ExternalInput").ap()
    pk1l = nc.dram_tensor("pack1l", [128, KP, PK1_W], BF16,
                          kind="ExternalInput").ap()
    ptf = nc.dram_tensor("ptailf", [4, PTF_W], F32, kind="ExternalInput").ap()
    ptb = nc.dram_tensor("ptailb", [1, PTB_W], BF16,
                         kind="ExternalInput").ap()
    pack2 = nc.dram_tensor("pack2", [128, KP * D], BF16,
                           kind="ExternalInput").ap()
    out = nc.dram_tensor("out", [BPC, D], F32, kind="ExternalOutput").ap()
    with tile.TileContext(nc) as tc:
        with ExitStack() as ctx:
            _emit(ctx, tc, [out], [src, pk1h, pk1l, ptf, ptb, pack2])
    nc.compile()
    return nc


_NC_CACHE = {}


def _get_nc(sparse: bool = True):
    if sparse not in _NC_CACHE:
        _NC_CACHE[sparse] = build_nc(sparse)
    return _NC_CACHE[sparse]


def pack_weights(target_shard, Wp, bp, Wa, ba, Vp, bv):
    """Build the packed weight arrays for one core."""
    import ml_dtypes
    f = np.float32
    bf = ml_dtypes.bfloat16

    wp = np.asarray(Wp, f)
    wp_hi = wp.astype(bf)
    wp_lo = (wp - wp_hi.astype(f)).astype(bf)
    tgt = np.asarray(target_shard, f)
    tgt_hi = tgt.astype(bf)
    tgt_lo = (tgt - tgt_hi.astype(f)).astype(bf)

    def chunked(wmat, tmat):
        w_r = wmat.reshape(KP, 128, D).transpose(1, 0, 2)
        t_r = tmat.T.reshape(KP, 128, BPC).transpose(1, 0, 2)
        return np.ascontiguousarray(
            np.concatenate([w_r, t_r], axis=2))       # [128, KP, 516] bf16

    pk1h = chunked(wp_hi, tgt_hi)
    pk1l = chunked(wp_lo, tgt_lo)
    ptf = np.concatenate(
        [np.broadcast_to(np.asarray(Vp, f).ravel()[None, :], (4, D)),
         np.full((4, 1), np.asarray(bv, f).ravel()[0], f)], axis=1)
    ptb = np.concatenate([np.asarray(bp, f).ravel(),
                          np.asarray(ba, f).ravel()])[None, :].astype(bf)
    pack2 = (np.asarray(Wa, f).reshape(KP, 128, D).transpose(1, 0, 2)
             .reshape(128, KP * D).astype(bf))
    return (pk1h, pk1l, np.ascontiguousarray(ptf), np.ascontiguousarray(ptb),
            np.ascontiguousarray(pack2))


def make_in_maps(source, target, Wp, bp, Wa, ba, Vp, bv):
    in_maps = []
    for c in range(N_CORES):
        bs = slice(c * BPC, (c + 1) * BPC)
        pk1h, pk1l, ptf, ptb, pack2 = pack_weights(
            target[bs], Wp, bp, Wa, ba, Vp, bv)
        in_maps.append({
            "source": np.ascontiguousarray(source[bs], dtype=np.float32),
            "pack1h": pk1h, "pack1l": pk1l, "ptailf": ptf, "ptailb": ptb,
            "pack2": pack2,
        })
    return in_maps


def kernel(source, target, Wp, bp, Wa, ba, Vp, bv, *, sparse=True, **run_kwargs):
    nc = _get_nc(sparse)
    in_maps = make_in_maps(source, target, Wp, bp, Wa, ba, Vp, bv)
    res = run_bass_kernel_spmd(nc, in_maps, core_ids=list(range(N_CORES)),
                               **run_kwargs)
    out = np.concatenate([r["out"] for r in res.results], axis=0)
    kernel.last_results = res
    return out
